# revision 4
# baseline (speedup 1.0000x reference)
"""Causal single-head attention (B=16, S=2048, D=1024, H=64) on 8 TRN2 cores.

Sharding: data-parallel over batch (2 per core); weights replicated.

Per-core Bass/Tile kernel, for each local batch:
  1. x is cast fp32->bf16 during the SWDGE load DMA, then transposed into
     xT [D-part, S] tiles via DMA-XBAR transpose (offloads transposition to
     the DMA engines; the PE never touches it).
  2. Projections on PE with packed weights [Wq/H | Wk] (M=128) and Wv:
     qT/kT/vT in [H, S] layout, which is exactly what the scores matmul
     needs (contraction over H on the partition dim).
  3. scoresT[sk, sq] per 128-wide key block, causal chunks only; exp() is
     applied by the ScalarE directly PSUM->SBUF(bf16).  No max-subtraction:
     scores = q.k/H are bounded (|s| < ~1) so exp never overflows, and
     softmax is shift-invariant so the result matches the reference.
     The diagonal block gets a multiplicative upper-triangular mask.
  4. out = attn @ [v | 1]: the appended ones-column accumulates the softmax
     denominator for free in PSUM; a reciprocal+scale normalizes at the end.
"""

import os
import sys

import numpy as np

if "/opt/trn_rl_repo" not in sys.path:
    sys.path.insert(0, "/opt/trn_rl_repo")

import concourse.bass as bass  # noqa: E402
import concourse.mybir as mybir  # noqa: E402
import concourse.tile as tile  # noqa: E402
from concourse import bacc  # noqa: E402
from concourse.bass_utils import run_bass_kernel_spmd  # noqa: E402
from concourse.masks import make_upper_triangular  # noqa: E402

F32 = mybir.dt.float32
BF16 = mybir.dt.bfloat16
AF = mybir.ActivationFunctionType

B, S, D, H = 16, 2048, 1024, 64
N_CORES = 8
B_PER_CORE = B // N_CORES


def _build_kernel(B_per_core: int, S: int, D: int, H: int,
                  scores_chunk: int = 1024):
    assert D % 128 == 0 and S % 512 == 0 and H == 64
    DC = D // 128          # d-chunks of 128
    ST = S // 128          # s-tiles of 128
    SC = S // 512          # s-chunks of 512
    KB = ST                # key blocks

    nc = bacc.Bacc("TRN2", target_bir_lowering=False, debug=False,
                   num_devices=N_CORES)
    x_in = nc.dram_tensor("x", [B_per_core, S, D], F32, kind="ExternalInput")
    wq_in = nc.dram_tensor("Wq", [D, H], F32, kind="ExternalInput")
    wk_in = nc.dram_tensor("Wk", [D, H], F32, kind="ExternalInput")
    wv_in = nc.dram_tensor("Wv", [D, H], F32, kind="ExternalInput")
    out_dram = nc.dram_tensor("out", [B_per_core, S, H], F32,
                              kind="ExternalOutput")

    with tile.TileContext(nc) as tc:
        with (
            tc.tile_pool(name="consts", bufs=1) as consts,
            tc.tile_pool(name="xbf", bufs=3) as xbf_pool,
            tc.tile_pool(name="xt", bufs=2) as xt_pool,
            tc.tile_pool(name="qkvt", bufs=2) as qkvt_pool,
            tc.tile_pool(name="vsb", bufs=2) as vsb_pool,
            tc.tile_pool(name="attnt", bufs=1) as attnt_pool,
            tc.tile_pool(name="outp", bufs=4) as out_pool,
            tc.tile_pool(name="pp", bufs=2, space="PSUM") as proj_psum,
            tc.tile_pool(name="sp", bufs=2, space="PSUM") as scores_psum,
            tc.tile_pool(name="ap", bufs=2, space="PSUM") as av_psum,
        ):
            # wqk: cols 0:64 = Wq * (1/H) (folds the score scale), 64:128 = Wk
            wqk = consts.tile([128, DC, 128], BF16)
            wv = consts.tile([128, DC, H], BF16)
            nc.gpsimd.dma_start(
                out=wqk[:, :, 0:H],
                in_=wq_in.rearrange("(c p) h -> p c h", p=128))
            nc.gpsimd.dma_start(
                out=wqk[:, :, H:128],
                in_=wk_in.rearrange("(c p) h -> p c h", p=128))
            nc.gpsimd.dma_start(
                out=wv[:],
                in_=wv_in.rearrange("(c p) h -> p c h", p=128))
            nc.vector.tensor_scalar_mul(wqk[:, :, 0:H], wqk[:, :, 0:H],
                                        1.0 / H)
            # mask[i, j] = 1.0 where j >= i (valid: sq_local >= sk_local)
            mask = consts.tile([128, 128], BF16)
            make_upper_triangular(nc, mask[:], val=1.0, diag=True)

            for b in range(B_per_core):
                # ---- load + cast + transpose x ----
                xt = xt_pool.tile([128, ST, DC, 128], BF16)
                for st in range(ST):
                    xbf = xbf_pool.tile([128, D], BF16)
                    nc.gpsimd.dma_start(
                        out=xbf[:], in_=x_in[b, st * 128:(st + 1) * 128, :])
                    # one XBAR transpose for the whole [128, D] tile: the
                    # 3D out's middle dim extends the partition dim, i.e.
                    # out[:, dc, :] = in_[:, dc*128:(dc+1)*128].T
                    nc.sync.dma_start(out=xt[:, st, :, :], in_=xbf[:],
                                      transpose=True)

                # ---- projections ----
                qT = qkvt_pool.tile([64, S], BF16, tag="qT")
                kT = qkvt_pool.tile([64, S], BF16, tag="kT")
                vT = qkvt_pool.tile([64, S], BF16, tag="vT")
                v_sb = vsb_pool.tile([128, KB, 80], BF16)
                # fill everything with 1.0; the v transposes below
                # overwrite cols 0:H, leaving col H == 1.0 (the
                # softmax-denominator accumulator column)
                nc.vector.memset(v_sb[:], 1.0)
                for sc in range(SC):
                    cs = slice(sc * 512, (sc + 1) * 512)
                    ps = proj_psum.tile([128, 512], F32, tag="proj")
                    st4 = slice(sc * 4, (sc + 1) * 4)
                    for dc in range(DC):
                        nc.tensor.matmul(ps[:], lhsT=wqk[:, dc, :],
                                         rhs=xt[:, st4, dc, :],
                                         start=(dc == 0), stop=(dc == DC - 1))
                    nc.vector.tensor_copy(qT[:, cs], ps[0:64, :])
                    nc.vector.tensor_copy(kT[:, cs], ps[64:128, :])
                    ps2 = proj_psum.tile([64, 512], F32, tag="proj")
                    for dc in range(DC):
                        nc.tensor.matmul(ps2[:], lhsT=wv[:, dc, :],
                                         rhs=xt[:, st4, dc, :],
                                         start=(dc == 0), stop=(dc == DC - 1))
                    nc.vector.tensor_copy(vT[:, cs], ps2[:])
                    nc.sync.dma_start(
                        out=v_sb[:, st4, 0:H],
                        in_=vT[:, cs], transpose=True)

                # ---- attention phase 1: attnT = exp(scoresT), causal ----
                attnT = attnt_pool.tile([128, KB, S], BF16)
                for kb in range(KB):
                    k0 = kb * 128
                    for base in range(k0, S, scores_chunk):
                        w = min(scores_chunk, S - base)
                        ps = scores_psum.tile([128, scores_chunk], F32,
                                              tag="scores")
                        for n0 in range(0, w, 512):
                            nw = min(512, w - n0)
                            nc.tensor.matmul(
                                ps[:, n0:n0 + nw],
                                lhsT=kT[:, k0:k0 + 128],
                                rhs=qT[:, base + n0:base + n0 + nw],
                                start=True, stop=True)
                        nc.scalar.activation(
                            out=attnT[:, kb, base:base + w],
                            in_=ps[:, 0:w], func=AF.Exp)
                    nc.vector.tensor_mul(
                        attnT[:, kb, k0:k0 + 128],
                        attnT[:, kb, k0:k0 + 128], mask[:])

                # ---- attention phase 2: out = (attn @ [v|1]) normalized ----
                for qb in range(ST):
                    po = av_psum.tile([128, H + 1], F32, tag="av")
                    q0 = qb * 128
                    for kb in range(qb + 1):
                        nc.tensor.matmul(
                            po[:],
                            lhsT=attnT[:, kb, q0:q0 + 128],
                            rhs=v_sb[:, kb, 0:H + 1],
                            start=(kb == 0), stop=(kb == qb))
                    recip = out_pool.tile([128, 1], F32, tag="recip")
                    nc.vector.reciprocal(recip[:], po[:, H:H + 1])
                    out_t = out_pool.tile([128, H], F32, tag="out")
                    nc.vector.tensor_scalar_mul(out_t[:], po[:, 0:H],
                                                recip[:])
                    nc.sync.dma_start(
                        out=out_dram[b, q0:q0 + 128, :], in_=out_t[:])

    nc.compile()
    return nc


_NC_CACHE = {}


def _get_nc():
    key = (B_PER_CORE, S, D, H)
    if key not in _NC_CACHE:
        _NC_CACHE[key] = _build_kernel(*key)
    return _NC_CACHE[key]


def kernel(x: np.ndarray, Wq: np.ndarray, Wk: np.ndarray, Wv: np.ndarray,
           _trace: bool = False):
    """Full-input entry point: shards over batch, runs 8 cores, gathers."""
    assert x.shape == (B, S, D)
    nc = _get_nc()
    core_ids = list(range(N_CORES))
    x = np.ascontiguousarray(np.asarray(x, dtype=np.float32))
    Wq = np.ascontiguousarray(np.asarray(Wq, dtype=np.float32))
    Wk = np.ascontiguousarray(np.asarray(Wk, dtype=np.float32))
    Wv = np.ascontiguousarray(np.asarray(Wv, dtype=np.float32))
    in_maps = [
        {"x": x[c * B_PER_CORE:(c + 1) * B_PER_CORE], "Wq": Wq, "Wk": Wk,
         "Wv": Wv}
        for c in core_ids
    ]
    res = run_bass_kernel_spmd(nc, in_maps, core_ids, trace=_trace)
    out = np.concatenate([res.results[c]["out"] for c in core_ids], axis=0)
    if _trace:
        return out, res
    return out
